# revision 2
# baseline (speedup 1.0000x reference)
"""Trainium2 Bass kernel for nn_DecoderBlock (BitNet-style decoder block
with self-attention, cross-attention and BitFeedForward), data-parallel
over (batch x sequence) tokens across 8 NeuronCores.

Sharding: 4096 tokens (B=2 x N=2048) split into 8 shards of 512 tokens;
cores 0-3 hold batch 0, cores 4-7 batch 1. Self-attention K/V are
computed on local tokens and AllGather-ed (one fused collective) within
each 4-core batch group; everything else is local with replicated
weights.

Implementation highlights (3.9x over the first working version):
- Weights are ternary-quantized host-side and shipped as bf16 [in, out]
  tiles plus a scale row (integer values are exact in bf16, so the
  bf16 matmuls reproduce the reference's quantized matmuls exactly up
  to fp32 accumulation); this removed ~1.2ms of on-device GpSimd/Vector
  weight prep and halves weight DMA traffic.
- Activation fake-quant: per-token absmax int8 in the integer domain.
  The RMS factor cancels out of the quant integers, so the rsqrt is
  only needed for the per-token dequant alpha, computed batched via one
  Ln+Exp pair per phase (Ln and Exp live in different ACT table sets;
  batching avoids ~90us of table reloads).
- One batched 3D-output DMA transpose per [128, F] tile emits the
  feature-major integer activations.
- Attention: head-paired score matmuls run as concurrent row-group
  pairs (tile_position (0,0)/(64,0), K=64 each); exp runs on 2-bank
  PSUM score tiles (double-buffered against the PE) with all weight
  scales folded into the per-partition exp scale; the softmax
  denominator rides the PV matmul as an appended ones-column.
- FFN GELUs are contiguous in ACT program order; gelu+quant+w2 are
  pipelined per token tile.
"""

import numpy as np
import ml_dtypes
from contextlib import ExitStack

import concourse.bacc as bacc
import concourse.mybir as mybir
import concourse.tile as tile
from concourse.bass_utils import run_bass_kernel_spmd
from concourse.masks import make_identity

F32 = mybir.dt.float32
BF16 = mybir.dt.bfloat16
I16 = mybir.dt.int16
AX = mybir.AxisListType
OP = mybir.AluOpType
ACT = mybir.ActivationFunctionType

# model dims
B, N, S, D = 2, 2048, 256, 768
HQ, HK, HEAD = 12, 6, 64
DKV = HEAD * HK          # 384
H4 = 4 * D               # 3072
NCORES = 8
GROUPS = [[0, 1, 2, 3], [4, 5, 6, 7]]
GSZ = 4
T = (B * N) // NCORES    # 512 tokens per core
NT = T // 128            # 4 token tiles per core
ST = S // 128            # 2 condition token tiles
KT = D // 128            # 6 feature tiles of D
KTH = H4 // 128          # 24 feature tiles of 4D
NSK = GSZ * NT           # 16 gathered key tiles (SA)

# head pairing for concurrent row-group score matmuls: q tile p holds
# heads (PERM[2p], PERM[2p+1]); PERM[2p] uses the low kv half, PERM[2p+1]
# the high half of k tile p//2.
PERM = [0, 2, 1, 3, 4, 6, 5, 7, 8, 10, 9, 11]

WSPECS = {
    'sa_wq': (D, D), 'sa_wk': (DKV, D), 'sa_wv': (DKV, D), 'sa_wo': (D, D),
    'ca_wq': (D, D), 'ca_wk': (DKV, D), 'ca_wv': (DKV, D), 'ca_wo': (D, D),
    'w_cond': (D, D), 'w1': (H4, D), 'w2': (D, H4),
}
WNAMES = list(WSPECS)
MIDX = {n: i for i, n in enumerate(WNAMES)}

_PROGRAM_CACHE = {}


class Ctx:
    pass


def _rsqrt(g, out, in_, eps_tile, tmp):
    """out = (in_ + eps)^-1/2 via exp(-0.5*ln(.)): stays in the
    natural_log_exp ACT table set shared with softmax's exp."""
    g.nc.scalar.activation(tmp, in_, ACT.Ln, bias=eps_tile)
    g.nc.scalar.activation(out, tmp, ACT.Exp, bias=0.0, scale=-0.5)


def _aq_stats(g, X, F, xqT, j, msx, amx, work):
    """Per-tile half of activation quant: stats + integer emit.
    The RMS factor cancels out of the quant integers
    (round(xn*127/absmax(xn)) == round(x*127/absmax(x))), so only alpha
    (via _aq_alphas) needs the rsqrt. Writes mean^2+var into msx[:, j],
    absmax into amx[:, j], integer-valued bf16 transposed into xqT."""
    nc, stat = g.nc, g.stat
    sub = 256 if F % 512 else 512
    ns = F // sub
    stats = stat.tile([128, ns, 6], F32, tag="bnst", name="bnst")
    Xg = X.rearrange("p (n s) -> p n s", s=sub)
    for gi in range(ns):
        nc.vector.bn_stats(stats[:, gi, :], Xg[:, gi, :])
    mv = stat.tile([128, 2], F32, tag="mv", name="mv")
    nc.vector.bn_aggr(mv, stats)
    ms = stat.tile([128, 1], F32, tag="s1", name="s1")
    nc.vector.tensor_mul(ms, mv[:, 0:1], mv[:, 0:1])
    nc.vector.tensor_add(msx[:, j:j + 1], ms, mv[:, 1:2])
    amax = stat.tile([128, 1], F32, tag="s4", name="s4")
    nc.vector.tensor_reduce(amax, X, axis=AX.X, op=OP.max,
                            apply_absolute_value=True)
    nc.vector.tensor_scalar_max(amx[:, j:j + 1], amax, 1e-12)
    ra = stat.tile([128, 1], F32, tag="s6", name="s6")
    nc.vector.reciprocal(ra, amx[:, j:j + 1])
    srnd = stat.tile([128, 1], F32, tag="s7", name="s7")
    nc.vector.tensor_scalar_mul(srnd, ra, 127.0)
    # fp32->int16 convert rounds to nearest on HW
    i16 = work.tile([128, F], I16, tag=f"i16_{F}", name=f"i16_{F}", bufs=2)
    nc.vector.tensor_scalar_mul(i16, X, srnd)
    xq = work.tile([128, F], BF16, tag=f"xqb_{F}", name=f"xqb_{F}", bufs=2)
    nc.scalar.copy(xq, i16)
    nc.sync.dma_start_transpose(xqT[:, :, j * 128:(j + 1) * 128], xq)


def _aq_alphas(g, msx, amx, amat, nj):
    """Batched alpha: amat = max(amx*rsqrt(msx+1e-6), 1e-5)/127.
    One Ln + one Exp instruction -> 2 ACT table loads per phase."""
    nc, stat = g.nc, g.stat
    lnm = stat.tile([128, nj], F32, tag="lnm", name="lnm")
    nc.scalar.activation(lnm, msx, ACT.Ln, bias=g.eps6)
    rr = stat.tile([128, nj], F32, tag="rr", name="rr")
    nc.scalar.activation(rr, lnm, ACT.Exp, bias=0.0, scale=-0.5)
    amn = stat.tile([128, nj], F32, tag="amn", name="amn")
    nc.vector.tensor_mul(amn, amx, rr)
    nc.vector.tensor_scalar_max(amn, amn, 1e-5)
    nc.vector.tensor_scalar_mul(amat, amn, 1.0 / 127.0)


def _act_quant_phase(g, tiles, F, xqT, amat, work):
    """Quantize a list of [128, F] tiles with batched alpha rsqrt."""
    nj = len(tiles)
    msx = g.stat.tile([128, nj], F32, tag=f"msx{nj}", name="msx")
    amx = g.stat.tile([128, nj], F32, tag=f"amx{nj}", name="amx")
    for j, X in enumerate(tiles):
        _aq_stats(g, X, F, xqT, j, msx, amx, work)
    _aq_alphas(g, msx, amx, amat, nj)


def _make_abcast(g, amat, nj, pool, tag):
    """[128, nj] per-token-tile alphas -> [128, nj*128] row broadcast."""
    nc, stat = g.nc, g.stat
    pst = g.psum.tile([nj, 128], F32, tag="ps", name="pst")
    nc.tensor.transpose(pst, amat, g.ident)
    at = stat.tile([nj, 128], F32, tag="at", name="at")
    nc.scalar.copy(at, pst)
    arow = stat.tile([1, nj * 128], F32, tag="arow", name="arow")
    for j in range(nj):
        nc.sync.dma_start(arow[0:1, j * 128:(j + 1) * 128], at[j:j + 1, :])
    ab = pool.tile([128, nj * 128], F32, tag=tag, name=tag)
    nc.gpsimd.partition_broadcast(ab, arow[0:1, :])
    return ab


def _layernorm(g, a_tiles, g_bc, b_bc, out_tiles):
    """LayerNorm with the per-tile rsqrt batched into one Ln+Exp pair."""
    nc, stat = g.nc, g.stat
    nj = len(a_tiles)
    mus = stat.tile([128, nj], F32, tag=f"lmu{nj}", name="lmu")
    vas = stat.tile([128, nj], F32, tag=f"lva{nj}", name="lva")
    for j, A in enumerate(a_tiles):
        stats = stat.tile([128, 3, 6], F32, tag="bnst", name="bnst")
        Ag = A.rearrange("p (n s) -> p n s", s=256)
        for gi in range(3):
            nc.vector.bn_stats(stats[:, gi, :], Ag[:, gi, :])
        mv = stat.tile([128, 2], F32, tag="mv", name="mv")
        nc.vector.bn_aggr(mv, stats)
        nc.vector.tensor_copy(mus[:, j:j + 1], mv[:, 0:1])
        nc.vector.tensor_copy(vas[:, j:j + 1], mv[:, 1:2])
    lnv = stat.tile([128, nj], F32, tag=f"lnv{nj}", name="lnv")
    nc.scalar.activation(lnv, vas, ACT.Ln, bias=g.eps5)
    rss = stat.tile([128, nj], F32, tag=f"rss{nj}", name="rss")
    nc.scalar.activation(rss, lnv, ACT.Exp, bias=0.0, scale=-0.5)
    for j, A in enumerate(a_tiles):
        X = out_tiles[j]
        nc.vector.tensor_scalar(X, A, mus[:, j:j + 1], rss[:, j:j + 1],
                                OP.subtract, OP.mult)
        nc.vector.tensor_mul(X, X, g_bc)
        nc.vector.tensor_add(X, X, b_bc)


def _attention(g, tc, n_s, kh, q_sb, va, mqk, a_out, work):
    """Paired-head GQA attention.
    kh: 3 tiles [128, n_s*128] bf16 (kv-pair feature-major, integer*alpha)
    q_sb: 6 tiles [128, T] bf16 (paired heads, integer*alpha)
    va: n_s tiles [128, HK, HEAD+1] bf16 (dequantized v + ones column)
    mqk: [128,1] fp32 m_q*m_k/sqrt(HEAD) exp scale
    a_out: NT tiles [128, D] bf16 token-major (permuted head order)."""
    nc, stat = g.nc, g.stat
    with tc.tile_pool(name="ps_s", bufs=2, space="PSUM") as pss_pool, \
         tc.tile_pool(name="ps_o", bufs=2, space="PSUM") as pso_pool:
        for p in range(6):
            t = p // 2
            ps_o = [pso_pool.tile([65, 512], F32, tag="pso",
                                  name=f"pso{r}") for r in range(2)]
            for s in range(n_s):
                ps_s = pss_pool.tile([128, 2, 512], F32, tag="pss",
                                     name="pss")
                for r in range(2):
                    nc.tensor.matmul(
                        ps_s[:, r, :],
                        kh[t][64 * r:64 * r + 64, 128 * s:128 * s + 128],
                        q_sb[p][64 * r:64 * r + 64, :],
                        start=True, stop=True,
                        tile_position=(64 * r, 0))
                p_sb = work.tile([128, 2, 512], BF16, tag="psb",
                                 name="psb", bufs=3)
                nc.scalar.activation(p_sb, ps_s, ACT.Exp, bias=0.0,
                                     scale=mqk)
                for r in range(2):
                    kv = 2 * (p // 2) + r
                    nc.tensor.matmul(ps_o[r], va[s][:, kv, :],
                                     p_sb[:, r, :],
                                     start=(s == 0),
                                     stop=(s == n_s - 1))
            for r in range(2):
                o_sb = work.tile([65, 512], F32, tag="osb", name="osb",
                                 bufs=2)
                nc.scalar.copy(o_sb, ps_o[r])
                hb = 2 * p + r
                for j in range(NT):
                    ps_t = g.psum.tile([128, 65], F32, tag="ps",
                                       name="ps_t")
                    nc.tensor.transpose(ps_t,
                                        o_sb[:, j * 128:(j + 1) * 128],
                                        g.ident[0:65, 0:65])
                    rec = stat.tile([128, 1], F32, tag="rec", name="rec")
                    nc.vector.reciprocal(rec, ps_t[:, 64:65])
                    nc.vector.tensor_scalar_mul(
                        a_out[j][:, 64 * hb:64 * hb + 64],
                        ps_t[:, 0:64], rec)


def build_program(groups=None):
    if groups is None:
        groups = GROUPS
    gsz = len(groups[0])
    n_s = gsz * NT
    nc = bacc.Bacc()

    x_in = nc.declare_dram_parameter("x_sh", [T, D], F32, isOutput=False)
    y_in = nc.declare_dram_parameter("y_b", [S, D], F32, isOutput=False)
    wt_in = {}
    for name, (O, I) in WSPECS.items():
        wt_in[name] = nc.declare_dram_parameter(f"{name}_t", [I, O], BF16,
                                                isOutput=False)
    mrow_in = nc.declare_dram_parameter("mrow", [1, 16], F32,
                                        isOutput=False)
    ln_in = {}
    for name in ('sa_g', 'sa_b', 'ca_g', 'ca_b'):
        ln_in[name] = nc.declare_dram_parameter(name, [D], F32,
                                                isOutput=False)
    out_sh = nc.declare_dram_parameter("out_sh", [T, D], F32, isOutput=True)

    g = Ctx()
    g.nc = nc

    with tile.TileContext(nc) as tc, ExitStack() as ctx:
        g.tc = tc
        const = ctx.enter_context(tc.tile_pool(name="const", bufs=1))
        g.const = const
        g.stat = ctx.enter_context(tc.tile_pool(name="stat", bufs=4))
        g.psum = ctx.enter_context(tc.tile_pool(name="psg", bufs=2,
                                                space="PSUM"))
        dram = ctx.enter_context(tc.tile_pool(name="dram", bufs=1,
                                              space="DRAM"))

        cc_in = dram.tile([2, DKV // 128, 128, T], BF16, name="cc_in")
        cc_out = dram.tile([gsz, 2, DKV // 128, 128, T], BF16,
                           name="cc_out")
        # v section viewed token-major [NT, 128, DKV]
        cc_in_v = cc_in[1, :, :, :].rearrange(
            "t p f -> (t p f)").rearrange("(j p f) -> j p f", p=128, f=DKV)

        g.eps6 = const.tile([128, 1], F32, name="eps6")
        nc.vector.memset(g.eps6, 1e-6)
        g.eps5 = const.tile([128, 1], F32, name="eps5")
        nc.vector.memset(g.eps5, 1e-5)
        g.ident = const.tile([128, 128], F32, name="ident")
        make_identity(nc, g.ident)

        mrow_sb = const.tile([1, 16], F32, name="mrow_sb")
        nc.sync.dma_start(mrow_sb, mrow_in[:, :])
        mb = const.tile([128, 16], F32, name="mb")
        nc.gpsimd.partition_broadcast(mb, mrow_sb[0:1, :])

        def mcol(name):
            i = MIDX[name]
            return mb[:, i:i + 1]

        ln_bc = {}
        for name in ('sa_g', 'sa_b', 'ca_g', 'ca_b'):
            row = const.tile([1, D], F32, tag=f"lnr_{name}",
                             name=f"lnr_{name}")
            nc.sync.dma_start(row[0:1, :],
                              ln_in[name][:].rearrange("(o d) -> o d", o=1))
            bc = const.tile([128, D], F32, tag=f"lnb_{name}",
                            name=f"lnb_{name}")
            nc.gpsimd.partition_broadcast(bc, row[0:1, :])
            ln_bc[name] = bc

        def load_w(name, pool):
            O, I = WSPECS[name]
            w = pool.tile([128, I // 128, O], BF16, tag=f"w_{name}",
                          name=f"w_{name}")
            nc.sync.dma_start(w, wt_in[name][:, :].rearrange(
                "(k p) o -> p k o", p=128))
            return w

        def proj_fm(wsb, xqT, Ttot, nmt, out_cb, psp):
            """feature-major projection: psum [128, Ttot] per mt tile."""
            for mt in range(nmt):
                ps = psp.tile([128, Ttot], F32, tag="pp", name="ps_fm")
                for k in range(KT):
                    nc.tensor.matmul(ps, wsb[:, k, mt * 128:(mt + 1) * 128],
                                     xqT[:, k, :], start=(k == 0),
                                     stop=(k == KT - 1))
                out_cb(mt, ps)

        def proj_tok(wsb, xqT, j, O, csz, out_cb, psp, nk=KT):
            """token-major projection for token tile j: psum [128, csz]
            per output chunk c."""
            for c in range(O // csz):
                ps = psp.tile([128, csz], F32, tag="pp", name="ps_tk")
                for k in range(nk):
                    nc.tensor.matmul(
                        ps, xqT[:, k, j * 128:(j + 1) * 128],
                        wsb[:, k, c * csz:(c + 1) * csz],
                        start=(k == 0), stop=(k == nk - 1))
                out_cb(c, ps)

        resid3 = ctx.enter_context(tc.tile_pool(name="resid3", bufs=1))
        x3 = [resid3.tile([128, D], F32, tag=f"x3_{j}", name=f"x3_{j}")
              for j in range(NT)]

        with tc.tile_pool(name="resid2", bufs=1) as resid2, \
             tc.tile_pool(name="ca_keep", bufs=1) as ca_keep, \
             tc.tile_pool(name="workA", bufs=1) as work:
            x2 = [resid2.tile([128, D], F32, tag=f"x2_{j}", name=f"x2_{j}")
                  for j in range(NT)]
            q_sb = []
            kch = []
            va_ca = []

            # ===== SA: projections + collective + CA precompute =====
            with tc.tile_pool(name="saw", bufs=1) as saw, \
                 tc.tile_pool(name="sa_xq", bufs=1) as sa_xq:
                with tc.tile_pool(name="caw1", bufs=1) as caw1, \
                     tc.tile_pool(name="ca_tmp", bufs=1) as ca_tmp, \
                     tc.tile_pool(name="ps_a", bufs=4,
                                  space="PSUM") as ps_a:
                    xqT = sa_xq.tile([128, KT, T], BF16, name="xqT")
                    amat_x = g.stat.tile([128, NT], F32, tag="amx",
                                         name="amx")
                    with tc.tile_pool(name="xload", bufs=1) as xload:
                        x_tiles = []
                        for j in range(NT):
                            xt = xload.tile([128, D], F32, tag=f"x_{j}",
                                            name=f"x_{j}")
                            nc.sync.dma_start(
                                xt, x_in[j * 128:(j + 1) * 128, :])
                            x_tiles.append(xt)
                        wq_sb = load_w('sa_wq', saw)
                        wk_sb = load_w('sa_wk', saw)
                        wv_sb = load_w('sa_wv', saw)
                        wo_sb = load_w('sa_wo', saw)
                        _act_quant_phase(g, x_tiles, D, xqT, amat_x, work)
                    abc_x = _make_abcast(g, amat_x, NT, sa_xq, "abcx")

                    # K proj (feature-major, alpha-only dequant) -> DRAM
                    def k_cb(mt, ps):
                        kl = work.tile([128, T], BF16, tag="klo",
                                       name="klo", bufs=2)
                        nc.vector.tensor_mul(kl, ps, abc_x)
                        nc.sync.dma_start(cc_in[0, mt, :, :], kl)
                    proj_fm(wk_sb, xqT, T, DKV // 128, k_cb, ps_a)

                    # V proj (token-major, full dequant) -> DRAM
                    for j in range(NT):
                        av = g.stat.tile([128, 1], F32, tag="s1",
                                         name="av")
                        nc.vector.tensor_mul(av, amat_x[:, j:j + 1],
                                             mcol('sa_wv'))

                        def v_cb(c, ps, j=j, av=av):
                            vt = work.tile([128, DKV], BF16, tag="vt",
                                           name="vt", bufs=2)
                            nc.scalar.activation(vt, ps, ACT.Copy,
                                                 bias=0.0, scale=av)
                            nc.sync.dma_start(cc_in_v[j, :, :], vt)
                        proj_tok(wv_sb, xqT, j, DKV, DKV, v_cb, ps_a)

                    nc.gpsimd.collective_compute(
                        "AllGather", OP.bypass, replica_groups=groups,
                        ins=[cc_in[:, :, :, :].opt()],
                        outs=[cc_out[:, :, :, :, :].opt()])

                    # Q proj (feature-major, alpha-only dequant)
                    def q_cb(mt, ps):
                        qt = sa_xq.tile([128, T], BF16, tag=f"q{mt}",
                                        name=f"q{mt}")
                        nc.vector.tensor_mul(qt, ps, abc_x)
                        q_sb.append(qt)
                    proj_fm(wq_sb, xqT, T, KT, q_cb, ps_a)

                    # ---- CA precompute (overlaps the collective) ----
                    wc_sb = load_w('w_cond', caw1)
                    wkc_sb = load_w('ca_wk', caw1)
                    wvc_sb = load_w('ca_wv', caw1)

                    y_tiles = []
                    for j in range(ST):
                        yt = ca_tmp.tile([128, D], F32, tag=f"y_{j}",
                                         name=f"y_{j}")
                        nc.sync.dma_start(
                            yt, y_in[j * 128:(j + 1) * 128, :])
                        y_tiles.append(yt)
                    yqT = ca_tmp.tile([128, KT, S], BF16, name="yqT")
                    amat_y = g.stat.tile([128, ST], F32, tag="amy",
                                         name="amy")
                    _act_quant_phase(g, y_tiles, D, yqT, amat_y, work)
                    yc = [ca_tmp.tile([128, D], F32, tag=f"yc_{j}",
                                      name=f"yc_{j}") for j in range(ST)]
                    for j in range(ST):
                        am = g.stat.tile([128, 1], F32, tag="s1",
                                         name="am")
                        nc.vector.tensor_mul(am, amat_y[:, j:j + 1],
                                             mcol('w_cond'))

                        def yc_cb(c, ps, j=j, am=am):
                            nc.vector.tensor_scalar_mul(
                                yc[j][:, c * 384:(c + 1) * 384], ps, am)
                        proj_tok(wc_sb, yqT, j, D, 384, yc_cb, ps_a)

                    ycqT = ca_tmp.tile([128, KT, S], BF16, name="ycqT")
                    amat_yc = g.stat.tile([128, ST], F32, tag="amyc",
                                          name="amyc")
                    _act_quant_phase(g, yc, D, ycqT, amat_yc, work)
                    abc_yc = _make_abcast(g, amat_yc, ST, ca_keep,
                                          "abcyc")

                    def kch_cb(mt, ps):
                        kc = ca_keep.tile([128, S], BF16, tag=f"kch{mt}",
                                          name=f"kch{mt}")
                        nc.vector.tensor_mul(kc, ps, abc_yc)
                        kch.append(kc)
                    proj_fm(wkc_sb, ycqT, S, DKV // 128, kch_cb, ps_a)

                    for j in range(ST):
                        avc = g.stat.tile([128, 1], F32, tag="s1",
                                          name="avc")
                        nc.vector.tensor_mul(avc, amat_yc[:, j:j + 1],
                                             mcol('ca_wv'))
                        vac = ca_keep.tile([128, HK, HEAD + 1], BF16,
                                           tag=f"vac{j}", name=f"vac{j}")

                        def vca_cb(c, ps, vac=vac, avc=avc):
                            nc.scalar.activation(
                                vac[:, :, 0:HEAD],
                                ps.rearrange("p (h e) -> p h e", e=HEAD),
                                ACT.Copy, bias=0.0, scale=avc)
                        proj_tok(wvc_sb, ycqT, j, DKV, DKV, vca_cb, ps_a)
                        nc.vector.memset(vac[:, :, HEAD:HEAD + 1], 1.0)
                        va_ca.append(vac)

                # ================= SA attention =================
                mqk_sa = const.tile([128, 1], F32, name="mqk_sa")
                nc.vector.tensor_mul(mqk_sa, mcol('sa_wq'), mcol('sa_wk'))
                with tc.tile_pool(name="sa_kv", bufs=1) as sa_kv, \
                     tc.tile_pool(name="sa_a", bufs=1) as sa_a:
                    kh = []
                    for t in range(DKV // 128):
                        kt = sa_kv.tile([128, n_s * 128], BF16,
                                        tag=f"kT{t}", name=f"kT{t}")
                        nc.sync.dma_start(
                            kt.rearrange("p (r f) -> p r f", r=gsz),
                            cc_out[:, 0, t, :, :].transpose([1, 0, 2]))
                        kh.append(kt)
                    va = []
                    for s in range(n_s):
                        r, j = s // NT, s % NT
                        vat = sa_kv.tile([128, HK, HEAD + 1], BF16,
                                         tag=f"va{s}", name=f"va{s}")
                        src = cc_out[r, 1, :, :, :].rearrange(
                            "t p f -> (t p f)").rearrange(
                            "(j p h e) -> j p h e", p=128, h=HK, e=HEAD)
                        nc.sync.dma_start(vat[:, :, 0:HEAD],
                                          src[j, :, :, :])
                        nc.vector.memset(vat[:, :, HEAD:HEAD + 1], 1.0)
                        va.append(vat)

                    a_tok = [sa_a.tile([128, D], BF16, tag=f"a{j}",
                                       name=f"a{j}") for j in range(NT)]
                    _attention(g, tc, n_s, kh, q_sb, va, mqk_sa, a_tok,
                               work)

                    ln_t = [sa_a.tile([128, D], BF16, tag=f"l{j}",
                                      name=f"l{j}") for j in range(NT)]
                    _layernorm(g, a_tok, ln_bc['sa_g'], ln_bc['sa_b'],
                               ln_t)
                    aqT = sa_a.tile([128, KT, T], BF16, name="aqT")
                    amat_a = g.stat.tile([128, NT], F32, tag="ama",
                                         name="ama")
                    _act_quant_phase(g, ln_t, D, aqT, amat_a, work)
                    # out proj + residual (x reloaded from DRAM)
                    with tc.tile_pool(name="ps_op", bufs=4,
                                      space="PSUM") as ps_op:
                        for j in range(NT):
                            xr = work.tile([128, D], F32, tag="xr",
                                           name="xr", bufs=2)
                            nc.sync.dma_start(
                                xr, x_in[j * 128:(j + 1) * 128, :])
                            ao = g.stat.tile([128, 1], F32, tag="s1",
                                             name="ao")
                            nc.vector.tensor_mul(ao, amat_a[:, j:j + 1],
                                                 mcol('sa_wo'))

                            def o_cb(c, ps, j=j, ao=ao, xr=xr):
                                nc.vector.scalar_tensor_tensor(
                                    x2[j][:, c * 384:(c + 1) * 384], ps,
                                    ao, xr[:, c * 384:(c + 1) * 384],
                                    OP.mult, OP.add)
                            proj_tok(wo_sb, aqT, j, D, 384, o_cb, ps_op)

            # ================= Phase CA =================
            with tc.tile_pool(name="caw2", bufs=1) as caw2, \
                 tc.tile_pool(name="ca_xq", bufs=1) as ca_xq, \
                 tc.tile_pool(name="ca_a", bufs=1) as ca_a:
                wqc_sb = load_w('ca_wq', caw2)
                woc_sb = load_w('ca_wo', caw2)
                x2qT = ca_xq.tile([128, KT, T], BF16, name="x2qT")
                amat_x2 = g.stat.tile([128, NT], F32, tag="amx2",
                                      name="amx2")
                _act_quant_phase(g, x2, D, x2qT, amat_x2, work)
                abc_x2 = _make_abcast(g, amat_x2, NT, ca_xq, "abcx2")
                q2_sb = []
                with tc.tile_pool(name="ps_ca", bufs=4,
                                  space="PSUM") as ps_ca:
                    def q2_cb(mt, ps):
                        qt = ca_xq.tile([128, T], BF16, tag=f"q2{mt}",
                                        name=f"q2{mt}")
                        nc.vector.tensor_mul(qt, ps, abc_x2)
                        q2_sb.append(qt)
                    proj_fm(wqc_sb, x2qT, T, KT, q2_cb, ps_ca)

                mqk_ca = const.tile([128, 1], F32, name="mqk_ca")
                nc.vector.tensor_mul(mqk_ca, mcol('ca_wq'), mcol('ca_wk'))
                a2_tok = [ca_a.tile([128, D], BF16, tag=f"a{j}",
                                    name=f"a{j}") for j in range(NT)]
                _attention(g, tc, ST, kch, q2_sb, va_ca, mqk_ca, a2_tok,
                           work)

                ln2 = [ca_a.tile([128, D], BF16, tag=f"l{j}",
                                 name=f"l{j}") for j in range(NT)]
                _layernorm(g, a2_tok, ln_bc['ca_g'], ln_bc['ca_b'], ln2)
                a2qT = ca_a.tile([128, KT, T], BF16, name="a2qT")
                amat_a2 = g.stat.tile([128, NT], F32, tag="ama2",
                                      name="ama2")
                _act_quant_phase(g, ln2, D, a2qT, amat_a2, work)
                with tc.tile_pool(name="ps_oc", bufs=4,
                                  space="PSUM") as ps_oc:
                    for j in range(NT):
                        ao = g.stat.tile([128, 1], F32, tag="s1",
                                         name="ao")
                        nc.vector.tensor_mul(ao, amat_a2[:, j:j + 1],
                                             mcol('ca_wo'))

                        def oc_cb(c, ps, j=j, ao=ao):
                            nc.vector.scalar_tensor_tensor(
                                x3[j][:, c * 384:(c + 1) * 384], ps, ao,
                                x2[j][:, c * 384:(c + 1) * 384],
                                OP.mult, OP.add)
                        proj_tok(woc_sb, a2qT, j, D, 384, oc_cb, ps_oc)

        # ================= Phase FFN =================
        with tc.tile_pool(name="w1p", bufs=1) as w1p, \
             tc.tile_pool(name="ffn_xq", bufs=1) as ffn_xq, \
             tc.tile_pool(name="workF", bufs=1) as workf, \
             tc.tile_pool(name="w2p", bufs=1) as w2p:
            w1_sb = load_w('w1', w1p)
            w2_sb = load_w('w2', w2p)
            x3qT = ffn_xq.tile([128, KT, T], BF16, name="x3qT")
            amat_x3 = g.stat.tile([128, NT], F32, tag="amx3", name="amx3")
            _act_quant_phase(g, x3, D, x3qT, amat_x3, workf)
            hqT = ffn_xq.tile([128, KTH, T], BF16, name="hqT")
            amat_h = g.stat.tile([128, NT], F32, tag="amh", name="amh")
            msx_h = g.stat.tile([128, NT], F32, tag="msxh", name="msxh")
            amx_h = g.stat.tile([128, NT], F32, tag="amxh", name="amxh")
            # per token tile: w1 matmuls + gelu, then quant stats --
            # pipelined so PE stays busy while DVE quantizes
            with tc.tile_pool(name="ps_w1", bufs=4,
                              space="PSUM") as ps_w1:
                for j in range(NT):
                    a3 = g.stat.tile([128, 1], F32, tag=f"a3_{j}",
                                     name=f"a3_{j}")
                    nc.vector.tensor_mul(a3, amat_x3[:, j:j + 1],
                                         mcol('w1'))
                    hw = workf.tile([128, H4], BF16, tag="hb", name="hb",
                                    bufs=2)

                    def h_cb(c, ps, hw=hw, a3=a3):
                        nc.scalar.activation(
                            hw[:, c * 512:(c + 1) * 512], ps,
                            ACT.Gelu, bias=0.0, scale=a3)
                    proj_tok(w1_sb, x3qT, j, H4, 512, h_cb, ps_w1)
                    _aq_stats(g, hw, H4, hqT, j, msx_h, amx_h, workf)
            _aq_alphas(g, msx_h, amx_h, amat_h, NT)
            with tc.tile_pool(name="ps_w2", bufs=4,
                              space="PSUM") as ps_w2:
                for j in range(NT):
                    ah = g.stat.tile([128, 1], F32, tag="s1", name="ah")
                    nc.vector.tensor_mul(ah, amat_h[:, j:j + 1],
                                         mcol('w2'))
                    xo = workf.tile([128, D], F32, tag="xo", name="xo",
                                    bufs=2)

                    def w2_cb(c, ps, j=j, ah=ah, xo=xo):
                        nc.vector.scalar_tensor_tensor(
                            xo[:, c * 384:(c + 1) * 384], ps, ah,
                            x3[j][:, c * 384:(c + 1) * 384],
                            OP.mult, OP.add)
                    proj_tok(w2_sb, hqT, j, D, 384, w2_cb, ps_w2,
                             nk=KTH)
                    nc.sync.dma_start(
                        out_sh[j * 128:(j + 1) * 128, :], xo)

    nc.finalize()
    return nc


def _get_program(key="full"):
    if key not in _PROGRAM_CACHE:
        _PROGRAM_CACHE[key] = build_program(
            GROUPS if key == "full" else [[0]])
    return _PROGRAM_CACHE[key]


LAST_RESULT = None


def _host_quant(w):
    w = np.asarray(w, np.float32)
    m = max(float(np.abs(w).mean()), 1e-5)
    wq = np.clip(np.rint(w / m), -1.0, 1.0).astype(np.float32)
    return wq, m


def kernel(**inputs):
    """Full-input entry: shard across 8 cores, run, gather."""
    global LAST_RESULT
    nc = _get_program()
    x = np.ascontiguousarray(np.asarray(inputs['x'], dtype=np.float32))
    y = np.ascontiguousarray(np.asarray(inputs['y'], dtype=np.float32))
    common = {}
    mrow = np.zeros((1, 16), np.float32)
    for name in WNAMES:
        wq, m = _host_quant(inputs[name])
        if name in ('sa_wq', 'ca_wq'):
            wq = wq.reshape(HQ, HEAD, D)[PERM].reshape(D, D)
            m = m / float(np.sqrt(HEAD))
        elif name in ('sa_wo', 'ca_wo'):
            wq = wq.reshape(D, HQ, HEAD)[:, PERM].reshape(D, D)
        mrow[0, MIDX[name]] = m
        common[f"{name}_t"] = np.ascontiguousarray(wq.T).astype(
            ml_dtypes.bfloat16)
    common["mrow"] = mrow
    for name in ('sa_g', 'sa_b', 'ca_g', 'ca_b'):
        v = np.asarray(inputs[name], np.float32)
        common[name] = np.ascontiguousarray(
            v.reshape(HQ, HEAD)[PERM].reshape(D))
    in_maps = []
    for c in range(NCORES):
        b, seg = c // GSZ, c % GSZ
        m = dict(common)
        m['x_sh'] = np.ascontiguousarray(x[b, seg * T:(seg + 1) * T, :])
        m['y_b'] = np.ascontiguousarray(y[b])
        in_maps.append(m)
    res = run_bass_kernel_spmd(nc, in_maps, core_ids=list(range(NCORES)))
    LAST_RESULT = res
    out = np.empty((B, N, D), np.float32)
    for c in range(NCORES):
        b, seg = c // GSZ, c % GSZ
        out[b, seg * T:(seg + 1) * T, :] = res.results[c]['out_sh']
    return out
